# revision 30
# baseline (speedup 1.0000x reference)
"""Bass/Tile Trainium2 kernel for a CLAPP SNN layer step, sharded row-wise
over the hidden dim across 8 NeuronCores.

Per core c (rows R = [512c, 512c+512)):
  cur[R]    = W_fc[R] @ inp           (DVE elementwise mult + ACT accum reduce)
  mem_new   = BETA*mem[R] + cur[R]
  spk[R]    = mem_new > 1.0 ; mem_out = mem_new - spk
  AllGather spk shards -> spk_full[4096]   (ncfw collective, HBM bounce)
  retro[R]  = W_retro[R] @ spk_full ; fb_new[R] = W_pred[R] @ spk_full
  dW_pred[R]= (LR*bf*feedback[R]) (x) spk_full
  dW_fc[R]  = a1[R] (x) inp + a2[R] (x) prev_inp
     a1 = LR*bf*feedback*surrogate(cur), a2 = bf*retro*surrogate(cur_prev)

Engine plan: DVE does the matvec multiplies (tensor_tensor fp32) and the
dW_fc combine; ScalarE does the free-dim reductions (activation Copy with
accum_out — tensor_tensor_reduce raises INTERNAL on hw) and the rank-1
outer products (Copy with per-partition scale). TensorE unused; this is a
memory-bound kernel.

All weight tiles share ONE 8-slot pool (same tag): W_fc's 8 tiles occupy
the slots first, so W_pred/W_retro prefetch is naturally throttled during
phase A and then streams through the collective window as slots free.
Broadcast x-vectors are loaded as replicated-read DMAs (AP.broadcast_to),
split in halves so the first matvec multiply starts as early as possible.

A warmup AllGather with the same shape as the real one (reading an unwritten
dummy buffer, so it fires immediately) runs at kernel start to absorb the
ncfw cold-start off the critical path.

DMA rings: sync HWDGE = W_fc/W_pred loads + dW stores; scalar HWDGE =
W_retro loads (separate FIFO); gpsimd SWDGE = broadcasts, small/scattered
transfers, collectives.

Hidden-dim shard vectors live in SBUF as [128, NB] with (p, b) = v[128b+p],
matching the weight-tile partition layout; the host pre/post-permutes them.
"""

import numpy as np

import concourse.bass as bass
import concourse.tile as tile
from concourse import bacc, mybir
from concourse.bass_utils import run_bass_kernel_spmd
from concourse.tile_rust import add_dep_helper

N_IN = 4096
N_HID = 4096
NCORES = 8
HS = N_HID // NCORES        # 512 rows per core
NB = HS // 128              # 4 row blocks per core
FT = 2048                   # free-dim tile for streamed weights
NF_IN = N_IN // FT          # 2
NF_HID = N_HID // FT        # 2

BETA = 0.95
LR = 2e-6
THRESH = 1.0
PI = float(np.pi)

F32 = mybir.dt.float32
ALU = mybir.AluOpType
AF = mybir.ActivationFunctionType


def _build_program(use_collective=True, warmup_collective=True):
    nc = bacc.Bacc(
        "TRN2", target_bir_lowering=False, debug=False, num_devices=NCORES
    )

    # ---- per-core DRAM I/O (shapes are the per-core shards) ----
    w_fc = nc.dram_tensor("w_fc", [HS, N_IN], F32, kind="ExternalInput")
    w_pred = nc.dram_tensor("w_pred", [HS, N_HID], F32, kind="ExternalInput")
    w_retro = nc.dram_tensor("w_retro", [HS, N_HID], F32, kind="ExternalInput")
    inp_d = nc.dram_tensor("inp", [1, N_IN], F32, kind="ExternalInput")
    pinp_d = nc.dram_tensor("prev_inp", [1, N_IN], F32, kind="ExternalInput")
    bf_d = nc.dram_tensor("bf", [1, 1], F32, kind="ExternalInput")
    # vectors arrive host-permuted as [128, NB]: (p, b) = v[128b + p]
    fb_d = nc.dram_tensor("feedback", [128, NB], F32, kind="ExternalInput")
    cprev_d = nc.dram_tensor("cur_prev", [128, NB], F32, kind="ExternalInput")
    mem_d = nc.dram_tensor("mem", [128, NB], F32, kind="ExternalInput")

    spk_o = nc.dram_tensor("spk_out", [128, NB], F32, kind="ExternalOutput")
    mem_o = nc.dram_tensor("mem_out", [128, NB], F32, kind="ExternalOutput")
    fbn_o = nc.dram_tensor("fb_new_out", [128, NB], F32, kind="ExternalOutput")
    dwp_o = nc.dram_tensor("dw_pred_out", [HS, N_HID], F32, kind="ExternalOutput")
    dwf_o = nc.dram_tensor("dw_fc_out", [HS, N_IN], F32, kind="ExternalOutput")

    with tile.TileContext(nc) as tc:
        with (
            tc.tile_pool(name="w_p", bufs=9) as w_p,
            tc.tile_pool(name="bcast_p", bufs=1) as bcast_p,
            tc.tile_pool(name="vec_p", bufs=1) as vec_p,
            tc.tile_pool(name="scr_p", bufs=2) as scr_p,
            tc.tile_pool(name="st1_p", bufs=2) as st1_p,
            tc.tile_pool(name="st2_p", bufs=2) as st2_p,
            tc.tile_pool(name="dwp_p", bufs=2) as dwp_p,
            tc.tile_pool(name="dram_p", bufs=1, space="DRAM") as dram_p,
        ):
            # ---- warmup collective, same shape as the real spk AllGather.
            # warm_in is never written -> no deps -> the trigger fires at
            # kernel start and ncfw cold-start overlaps phase A. ----
            warm_cc = None
            if use_collective and warmup_collective:
                warm_in = dram_p.tile([HS], F32, name="warm_in")
                warm_out = dram_p.tile([N_HID], F32, name="warm_out",
                                       addr_space="Shared")
                warm_cc = nc.gpsimd.collective_compute(
                    "AllGather",
                    ALU.bypass,
                    ins=[warm_in.opt()],
                    outs=[warm_out.opt()],
                    replica_groups=[list(range(NCORES))],
                )

            # ---- broadcast + small loads (SWDGE ring) ----
            inp_bc = []
            for h in range(NF_IN):
                t = bcast_p.tile([128, FT], F32, name=f"inp_bc{h}")
                ld = nc.gpsimd.dma_start(
                    t[:],
                    inp_d.ap()[:, FT * h : FT * (h + 1)].broadcast_to([128, FT]),
                )
                if warm_cc is not None and h == 0:
                    # order-only edge: the warmup doorbell goes FIRST on the
                    # gpsimd stream so ncfw warms while phase A streams
                    add_dep_helper(ld.ins, warm_cc.ins, sync=False,
                                   reason="warmup collective first on gpsimd")
                inp_bc.append(t)
            bf_col = vec_p.tile([128, 1], F32)
            nc.gpsimd.dma_start(bf_col[:], bf_d.ap().broadcast_to([128, 1]))
            fb_t = vec_p.tile([128, NB], F32)
            nc.gpsimd.dma_start(fb_t[:], fb_d.ap())
            cprev_t = vec_p.tile([128, NB], F32)
            nc.gpsimd.dma_start(cprev_t[:], cprev_d.ap())
            mem_t = vec_p.tile([128, NB], F32)
            nc.gpsimd.dma_start(mem_t[:], mem_d.ap())

            def matvec_block(w_dram, dma_eng, tname, bc_tiles, b, acc_col,
                             gate=None):
                """acc_col[128,1] = sum_f W[128b:128b+128, f] * x[f].
                Returns the final accumulate instruction. When `gate` is an
                instruction, the weight loads get a dep on it so prefetch
                doesn't steal HBM bandwidth from earlier critical loads."""
                parts = vec_p.tile([128, NF_IN], F32, name=f"parts_{tname}_{b}")
                for h in range(NF_IN):
                    wt = w_p.tile([128, FT], F32, name="wt", tag="wt")
                    ld = dma_eng.dma_start(
                        wt[:],
                        w_dram.ap()[128 * b : 128 * (b + 1), FT * h : FT * (h + 1)],
                    )
                    if gate is not None:
                        # first arg waits on second: the load waits for `gate`
                        add_dep_helper(ld.ins, gate.ins,
                                       reason="hold phase-B weight prefetch")
                    scr = scr_p.tile([128, FT], F32, name="scr")
                    nc.vector.tensor_mul(scr[:], wt[:], bc_tiles[h][:])
                    # in-place Copy: only accum_out matters (hw-verified ok)
                    nc.scalar.activation(
                        scr[:], scr[:], AF.Copy, bias=0.0, scale=1.0,
                        accum_out=parts[:, h : h + 1],
                    )
                return nc.vector.tensor_add(acc_col, parts[:, 0:1], parts[:, 1:2])

            # ---- phase A: cur = W_fc @ inp ----
            cur = vec_p.tile([128, NB], F32)
            cur_adds = []
            for b in range(NB):
                cur_adds.append(
                    matvec_block(w_fc, nc.sync, "fc", inp_bc, b,
                                 cur[:, b : b + 1])
                )
            prefetch_gate = cur_adds[1]

            # ---- LIF update + spike ----
            mem_new = vec_p.tile([128, NB], F32)
            nc.vector.tensor_scalar_mul(mem_new[:], mem_t[:], BETA)
            nc.vector.tensor_add(mem_new[:], mem_new[:], cur[:])
            spk = vec_p.tile([128, NB], F32)
            nc.vector.tensor_scalar(spk[:], mem_new[:], THRESH, None, ALU.is_gt)
            memo = vec_p.tile([128, NB], F32)
            nc.vector.tensor_sub(memo[:], mem_new[:], spk[:])
            nc.gpsimd.dma_start(spk_o.ap(), spk[:])
            nc.gpsimd.dma_start(mem_o.ap(), memo[:])

            # ---- surrogate(cur) and a1 (available before the collective) ----
            def surrogate(dst, x, nm):
                s = vec_p.tile([128, NB], F32, name=f"surr_s_{nm}")
                nc.vector.tensor_scalar_mul(s[:], x[:], PI)
                nc.vector.tensor_mul(s[:], s[:], s[:])
                # den = pi*(1 + s^2) = s^2*pi + pi
                nc.vector.tensor_scalar(s[:], s[:], PI, PI, ALU.mult, ALU.add)
                nc.vector.reciprocal(dst[:], s[:])

            surr_c = vec_p.tile([128, NB], F32)
            surrogate(surr_c, cur, "c")
            surr_p = vec_p.tile([128, NB], F32)
            surrogate(surr_p, cprev_t, "p")
            a1 = vec_p.tile([128, NB], F32)   # LR*bf*feedback*surr(cur)
            nc.vector.tensor_mul(a1[:], fb_t[:], surr_c[:])
            nc.vector.tensor_scalar(a1[:], a1[:], bf_col[:], LR, ALU.mult,
                                    ALU.mult)
            a0 = vec_p.tile([128, NB], F32)   # LR*bf*feedback
            nc.vector.tensor_scalar(a0[:], fb_t[:], bf_col[:], LR, ALU.mult,
                                    ALU.mult)

            # ---- AllGather spk across the 8 cores ----
            spk_bc = []
            if use_collective:
                spk_cc_in = dram_p.tile([HS], F32, name="spk_cc_in")
                # DRAM side in true row order: index 128b + p <- tile (p, b)
                nc.gpsimd.dma_start(
                    spk_cc_in.rearrange("(b p) -> p b", p=128), spk[:]
                )
                spk_cc_out = dram_p.tile([N_HID], F32, name="spk_cc_out",
                                         addr_space="Shared")
                nc.gpsimd.collective_compute(
                    "AllGather",
                    ALU.bypass,
                    ins=[spk_cc_in.opt()],
                    outs=[spk_cc_out.opt()],
                    replica_groups=[list(range(NCORES))],
                )
                spk_row = spk_cc_out.rearrange("(r f) -> r f", r=1)
                # two rings in parallel so both halves land ~simultaneously
                for h, eng in zip(range(NF_HID), (nc.gpsimd, nc.scalar)):
                    t = bcast_p.tile([128, FT], F32, name=f"spk_bc{h}")
                    eng.dma_start(
                        t[:],
                        spk_row[:, FT * h : FT * (h + 1)].broadcast_to([128, FT]),
                    )
                    spk_bc.append(t)
            else:
                for h in range(NF_HID):
                    t = bcast_p.tile([128, FT], F32, name=f"spk_bc{h}")
                    nc.vector.memset(t[:], 1.0)
                    spk_bc.append(t)

            # pinp_bc loads fill the collective window (only needed by t2,
            # deep in phase B)
            pinp_bc = []
            for h in range(NF_IN):
                t = bcast_p.tile([128, FT], F32, name=f"pinp_bc{h}")
                ld = nc.gpsimd.dma_start(
                    t[:],
                    pinp_d.ap()[:, FT * h : FT * (h + 1)].broadcast_to([128, FT]),
                )
                add_dep_helper(ld.ins, prefetch_gate.ins,
                               reason="hold pinp bcast until phase A tail")
                pinp_bc.append(t)

            # ---- dW_fc term 1 (a1 (x) inp): ACT work that can overlap the
            # collective window ----
            t1s = []
            for b in range(NB):
                for h in range(NF_IN):
                    t1 = st1_p.tile([128, FT], F32, name="t1")
                    nc.scalar.activation(
                        t1[:], inp_bc[h][:], AF.Copy,
                        bias=0.0, scale=a1[:, b : b + 1],
                    )
                    t1s.append(t1)

            # ---- phase B matvecs: retro first (unblocks dW_fc), then pred ----
            fbn = vec_p.tile([128, NB], F32)
            retro = vec_p.tile([128, NB], F32)
            for b in range(NB):
                matvec_block(w_retro, nc.sync, "retro", spk_bc, b,
                             retro[:, b : b + 1], gate=prefetch_gate)

            a2 = vec_p.tile([128, NB], F32)   # bf*retro*surr(cur_prev)
            nc.vector.tensor_mul(a2[:], retro[:], surr_p[:])
            nc.vector.tensor_scalar(a2[:], a2[:], bf_col[:], None, ALU.mult)

            # ---- dW_fc = t1 + a2 (x) prev_inp (emitted before the pred
            # matvecs so its stores start mid-phase-B) ----
            for b in range(NB):
                for h in range(NF_IN):
                    t1 = t1s[b * NF_IN + h]
                    t2 = st2_p.tile([128, FT], F32, name="t2")
                    nc.vector.tensor_scalar(
                        t2[:], pinp_bc[h][:], a2[:, b : b + 1], None, ALU.mult
                    )
                    nc.vector.tensor_add(t2[:], t2[:], t1[:])
                    nc.sync.dma_start(
                        dwf_o.ap()[128 * b : 128 * (b + 1), FT * h : FT * (h + 1)],
                        t2[:],
                    )

            for b in range(NB):
                matvec_block(w_pred, nc.sync, "pred", spk_bc, b,
                             fbn[:, b : b + 1], gate=prefetch_gate)
            nc.gpsimd.dma_start(fbn_o.ap(), fbn[:])

            # ---- dW_pred = a0 (x) spk_full; outers split ACT/DVE ----
            for b in range(NB):
                for h in range(NF_HID):
                    dwp = dwp_p.tile([128, FT], F32, name="dwp")
                    if b % 2 == 0:
                        nc.scalar.activation(
                            dwp[:], spk_bc[h][:], AF.Copy,
                            bias=0.0, scale=a0[:, b : b + 1],
                        )
                    else:
                        nc.vector.tensor_scalar(
                            dwp[:], spk_bc[h][:], a0[:, b : b + 1], None,
                            ALU.mult,
                        )
                    nc.sync.dma_start(
                        dwp_o.ap()[128 * b : 128 * (b + 1), FT * h : FT * (h + 1)],
                        dwp[:],
                    )

    nc.compile()
    return nc


_NC_CACHE = None


def _get_program():
    global _NC_CACHE
    if _NC_CACHE is None:
        _NC_CACHE = _build_program()
    return _NC_CACHE


def _swz(v):
    # [HS] -> [128, NB] with (p, b) = v[128b + p]
    return np.ascontiguousarray(v.reshape(NB, 128).T)


def _unswz(m):
    # [128, NB] -> [HS]
    return np.ascontiguousarray(m.T).reshape(-1)


def kernel(inp, bf, W_fc, W_pred, W_retro, feedback, cur_prev, prev_inp, mem):
    inp = np.asarray(inp, np.float32)
    bf = np.asarray(bf, np.float32)
    W_fc = np.asarray(W_fc, np.float32)
    W_pred = np.asarray(W_pred, np.float32)
    W_retro = np.asarray(W_retro, np.float32)
    feedback = np.asarray(feedback, np.float32)
    cur_prev = np.asarray(cur_prev, np.float32)
    prev_inp = np.asarray(prev_inp, np.float32)
    mem = np.asarray(mem, np.float32)

    nc = _get_program()

    in_maps = []
    for c in range(NCORES):
        r = slice(HS * c, HS * (c + 1))
        in_maps.append({
            "w_fc": np.ascontiguousarray(W_fc[r]),
            "w_pred": np.ascontiguousarray(W_pred[r]),
            "w_retro": np.ascontiguousarray(W_retro[r]),
            "inp": inp.reshape(1, N_IN),
            "prev_inp": prev_inp.reshape(1, N_IN),
            "bf": bf.reshape(1, 1),
            "feedback": _swz(feedback[r]),
            "cur_prev": _swz(cur_prev[r]),
            "mem": _swz(mem[r]),
        })

    res = run_bass_kernel_spmd(nc, in_maps, list(range(NCORES))).results

    spk = np.concatenate([_unswz(res[c]["spk_out"]) for c in range(NCORES)])
    mem_out = np.concatenate([_unswz(res[c]["mem_out"]) for c in range(NCORES)])
    fb_new = np.concatenate([_unswz(res[c]["fb_new_out"]) for c in range(NCORES)])
    dw_pred = np.concatenate([res[c]["dw_pred_out"] for c in range(NCORES)], axis=0)
    dw_fc = np.concatenate([res[c]["dw_fc_out"] for c in range(NCORES)], axis=0)
    return spk, mem_out, dw_pred, dw_fc, fb_new


# revision 33
# speedup vs baseline: 1.1150x; 1.1150x over previous
"""Bass/Tile Trainium2 kernel for a CLAPP SNN layer step, sharded row-wise
over the hidden dim across 8 NeuronCores.

Per core c (rows R = [512c, 512c+512)):
  cur[R]    = W_fc[R] @ inp           (DVE elementwise mult + ACT accum reduce)
  mem_new   = BETA*mem[R] + cur[R]
  spk[R]    = mem_new > 1.0 ; mem_out = mem_new - spk
  AllGather spk shards -> spk_full[4096]   (ncfw collective, HBM bounce)
  retro[R]  = W_retro[R] @ spk_full ; fb_new[R] = W_pred[R] @ spk_full
  dW_pred[R]= (LR*bf*feedback[R]) (x) spk_full
  dW_fc[R]  = a1[R] (x) inp + a2[R] (x) prev_inp
     a1 = LR*bf*feedback*surrogate(cur), a2 = bf*retro*surrogate(cur_prev)

Engine plan: DVE does the matvec multiplies (tensor_tensor fp32) and the
dW_fc combine; ScalarE does the free-dim reductions (activation Copy with
accum_out — tensor_tensor_reduce raises INTERNAL on hw) and the rank-1
outer products (Copy with per-partition scale). TensorE unused; this is a
memory-bound kernel.

All weight tiles share ONE 8-slot pool (same tag): W_fc's 8 tiles occupy
the slots first, so W_pred/W_retro prefetch is naturally throttled during
phase A and then streams through the collective window as slots free.
Broadcast x-vectors are loaded as replicated-read DMAs (AP.broadcast_to),
split in halves so the first matvec multiply starts as early as possible.

A warmup AllGather with the same shape as the real one (reading an unwritten
dummy buffer, so it fires immediately) runs at kernel start to absorb the
ncfw cold-start off the critical path.

DMA rings: sync HWDGE = W_fc/W_pred loads + dW stores; scalar HWDGE =
W_retro loads (separate FIFO); gpsimd SWDGE = broadcasts, small/scattered
transfers, collectives.

Hidden-dim shard vectors live in SBUF as [128, NB] with (p, b) = v[128b+p],
matching the weight-tile partition layout; the host pre/post-permutes them.
"""

import numpy as np

import concourse.bass as bass
import concourse.tile as tile
from concourse import bacc, mybir
from concourse.bass_utils import run_bass_kernel_spmd
from concourse.tile_rust import add_dep_helper

N_IN = 4096
N_HID = 4096
NCORES = 8
HS = N_HID // NCORES        # 512 rows per core
NB = HS // 128              # 4 row blocks per core
FT = 2048                   # free-dim tile for streamed weights
NF_IN = N_IN // FT          # 2
NF_HID = N_HID // FT        # 2

BETA = 0.95
LR = 2e-6
THRESH = 1.0
PI = float(np.pi)

F32 = mybir.dt.float32
ALU = mybir.AluOpType
AF = mybir.ActivationFunctionType


def _build_program(use_collective=True, warmup_collective=True):
    nc = bacc.Bacc(
        "TRN2", target_bir_lowering=False, debug=False, num_devices=NCORES
    )

    # ---- per-core DRAM I/O (shapes are the per-core shards) ----
    w_fc = nc.dram_tensor("w_fc", [HS, N_IN], F32, kind="ExternalInput")
    w_pred = nc.dram_tensor("w_pred", [HS, N_HID], F32, kind="ExternalInput")
    w_retro = nc.dram_tensor("w_retro", [HS, N_HID], F32, kind="ExternalInput")
    inp_d = nc.dram_tensor("inp", [1, N_IN], F32, kind="ExternalInput")
    pinp_d = nc.dram_tensor("prev_inp", [1, N_IN], F32, kind="ExternalInput")
    bf_d = nc.dram_tensor("bf", [1, 1], F32, kind="ExternalInput")
    # vectors arrive host-permuted as [128, NB]: (p, b) = v[128b + p]
    fb_d = nc.dram_tensor("feedback", [128, NB], F32, kind="ExternalInput")
    cprev_d = nc.dram_tensor("cur_prev", [128, NB], F32, kind="ExternalInput")
    mem_d = nc.dram_tensor("mem", [128, NB], F32, kind="ExternalInput")

    spk_o = nc.dram_tensor("spk_out", [128, NB], F32, kind="ExternalOutput")
    mem_o = nc.dram_tensor("mem_out", [128, NB], F32, kind="ExternalOutput")
    fbn_o = nc.dram_tensor("fb_new_out", [128, NB], F32, kind="ExternalOutput")
    dwp_o = nc.dram_tensor("dw_pred_out", [HS, N_HID], F32, kind="ExternalOutput")
    dwf_o = nc.dram_tensor("dw_fc_out", [HS, N_IN], F32, kind="ExternalOutput")

    with tile.TileContext(nc) as tc:
        with (
            tc.tile_pool(name="w_p", bufs=8) as w_p,
            tc.tile_pool(name="bcast_p", bufs=1) as bcast_p,
            tc.tile_pool(name="vec_p", bufs=1) as vec_p,
            tc.tile_pool(name="scr_p", bufs=2) as scr_p,
            tc.tile_pool(name="dump_p", bufs=1) as dump_p,
            tc.tile_pool(name="st1_p", bufs=2) as st1_p,
            tc.tile_pool(name="st2_p", bufs=2) as st2_p,
            tc.tile_pool(name="dwp_p", bufs=2) as dwp_p,
            tc.tile_pool(name="dram_p", bufs=1, space="DRAM") as dram_p,
        ):
            # ---- warmup collective, same shape as the real spk AllGather.
            # warm_in is never written -> no deps -> the trigger fires at
            # kernel start and ncfw cold-start overlaps phase A. ----
            warm_cc = None
            if use_collective and warmup_collective:
                warm_in = dram_p.tile([HS], F32, name="warm_in")
                warm_out = dram_p.tile([N_HID], F32, name="warm_out",
                                       addr_space="Shared")
                warm_cc = nc.gpsimd.collective_compute(
                    "AllGather",
                    ALU.bypass,
                    ins=[warm_in.opt()],
                    outs=[warm_out.opt()],
                    replica_groups=[list(range(NCORES))],
                )

            # ---- broadcast + small loads (SWDGE ring) ----
            inp_bc = []
            for h in range(NF_IN):
                t = bcast_p.tile([128, FT], F32, name=f"inp_bc{h}")
                ld = nc.gpsimd.dma_start(
                    t[:],
                    inp_d.ap()[:, FT * h : FT * (h + 1)].broadcast_to([128, FT]),
                )
                if warm_cc is not None and h == 0:
                    # order-only edge: the warmup doorbell goes FIRST on the
                    # gpsimd stream so ncfw warms while phase A streams
                    add_dep_helper(ld.ins, warm_cc.ins, sync=False,
                                   reason="warmup collective first on gpsimd")
                inp_bc.append(t)
            bf_col = vec_p.tile([128, 1], F32)
            nc.gpsimd.dma_start(bf_col[:], bf_d.ap().broadcast_to([128, 1]))
            fb_t = vec_p.tile([128, NB], F32)
            nc.gpsimd.dma_start(fb_t[:], fb_d.ap())
            cprev_t = vec_p.tile([128, NB], F32)
            nc.gpsimd.dma_start(cprev_t[:], cprev_d.ap())
            mem_t = vec_p.tile([128, NB], F32)
            nc.gpsimd.dma_start(mem_t[:], mem_d.ap())

            # one shared dump target for every ACT reduce (ACT is serial)
            dump = dump_p.tile([128, FT], F32)

            def matvec_block(w_dram, dma_eng, tname, bc_tiles, b, acc_col,
                             gate=None):
                """acc_col[128,1] = sum_f W[128b:128b+128, f] * x[f].
                Returns the final accumulate instruction. When `gate` is an
                instruction, the weight loads get a dep on it so prefetch
                doesn't steal HBM bandwidth from earlier critical loads."""
                parts = vec_p.tile([128, NF_IN], F32, name=f"parts_{tname}_{b}")
                for h in range(NF_IN):
                    wt = w_p.tile([128, FT], F32, name="wt", tag="wt")
                    ld = dma_eng.dma_start(
                        wt[:],
                        w_dram.ap()[128 * b : 128 * (b + 1), FT * h : FT * (h + 1)],
                    )
                    if gate is not None:
                        # first arg waits on second: the load waits for `gate`
                        add_dep_helper(ld.ins, gate.ins,
                                       reason="hold phase-B weight prefetch")
                    scr = scr_p.tile([128, FT], F32, name="scr")
                    nc.vector.tensor_mul(scr[:], wt[:], bc_tiles[h][:])
                    nc.scalar.activation(
                        dump[:], scr[:], AF.Copy, bias=0.0, scale=1.0,
                        accum_out=parts[:, h : h + 1],
                    )
                return nc.vector.tensor_add(acc_col, parts[:, 0:1], parts[:, 1:2])

            # ---- phase A: cur = W_fc @ inp ----
            cur = vec_p.tile([128, NB], F32)
            cur_adds = []
            for b in range(NB):
                cur_adds.append(
                    matvec_block(w_fc, nc.sync, "fc", inp_bc, b,
                                 cur[:, b : b + 1])
                )
            prefetch_gate = cur_adds[1]

            # ---- LIF update + spike ----
            mem_new = vec_p.tile([128, NB], F32)
            nc.vector.tensor_scalar_mul(mem_new[:], mem_t[:], BETA)
            nc.vector.tensor_add(mem_new[:], mem_new[:], cur[:])
            spk = vec_p.tile([128, NB], F32)
            nc.vector.tensor_scalar(spk[:], mem_new[:], THRESH, None, ALU.is_gt)
            memo = vec_p.tile([128, NB], F32)
            nc.vector.tensor_sub(memo[:], mem_new[:], spk[:])
            nc.gpsimd.dma_start(spk_o.ap(), spk[:])
            nc.gpsimd.dma_start(mem_o.ap(), memo[:])

            # ---- surrogate(cur) and a1 (available before the collective) ----
            def surrogate(dst, x, nm):
                s = vec_p.tile([128, NB], F32, name=f"surr_s_{nm}")
                nc.vector.tensor_scalar_mul(s[:], x[:], PI)
                nc.vector.tensor_mul(s[:], s[:], s[:])
                # den = pi*(1 + s^2) = s^2*pi + pi
                nc.vector.tensor_scalar(s[:], s[:], PI, PI, ALU.mult, ALU.add)
                nc.vector.reciprocal(dst[:], s[:])

            surr_c = vec_p.tile([128, NB], F32)
            surrogate(surr_c, cur, "c")
            surr_p = vec_p.tile([128, NB], F32)
            surrogate(surr_p, cprev_t, "p")
            a1 = vec_p.tile([128, NB], F32)   # LR*bf*feedback*surr(cur)
            nc.vector.tensor_mul(a1[:], fb_t[:], surr_c[:])
            nc.vector.tensor_scalar(a1[:], a1[:], bf_col[:], LR, ALU.mult,
                                    ALU.mult)
            a0 = vec_p.tile([128, NB], F32)   # LR*bf*feedback
            nc.vector.tensor_scalar(a0[:], fb_t[:], bf_col[:], LR, ALU.mult,
                                    ALU.mult)

            # ---- AllGather spk across the 8 cores ----
            spk_bc = []
            if use_collective:
                spk_cc_in = dram_p.tile([HS], F32, name="spk_cc_in")
                # DRAM side in true row order: index 128b + p <- tile (p, b)
                nc.gpsimd.dma_start(
                    spk_cc_in.rearrange("(b p) -> p b", p=128), spk[:]
                )
                spk_cc_out = dram_p.tile([N_HID], F32, name="spk_cc_out",
                                         addr_space="Shared")
                nc.gpsimd.collective_compute(
                    "AllGather",
                    ALU.bypass,
                    ins=[spk_cc_in.opt()],
                    outs=[spk_cc_out.opt()],
                    replica_groups=[list(range(NCORES))],
                )
                spk_row = spk_cc_out.rearrange("(r f) -> r f", r=1)
                # two rings in parallel so both halves land ~simultaneously
                for h, eng in zip(range(NF_HID), (nc.gpsimd, nc.scalar)):
                    t = bcast_p.tile([128, FT], F32, name=f"spk_bc{h}")
                    eng.dma_start(
                        t[:],
                        spk_row[:, FT * h : FT * (h + 1)].broadcast_to([128, FT]),
                    )
                    spk_bc.append(t)
            else:
                for h in range(NF_HID):
                    t = bcast_p.tile([128, FT], F32, name=f"spk_bc{h}")
                    nc.vector.memset(t[:], 1.0)
                    spk_bc.append(t)

            # pinp_bc loads fill the collective window (only needed by t2,
            # deep in phase B)
            pinp_bc = []
            for h in range(NF_IN):
                t = bcast_p.tile([128, FT], F32, name=f"pinp_bc{h}")
                ld = nc.gpsimd.dma_start(
                    t[:],
                    pinp_d.ap()[:, FT * h : FT * (h + 1)].broadcast_to([128, FT]),
                )
                add_dep_helper(ld.ins, prefetch_gate.ins,
                               reason="hold pinp bcast until phase A tail")
                pinp_bc.append(t)

            # ---- dW_fc term 1 (a1 (x) inp): ACT work that can overlap the
            # collective window ----
            t1s = []
            for b in range(NB):
                for h in range(NF_IN):
                    t1 = st1_p.tile([128, FT], F32, name="t1")
                    nc.scalar.activation(
                        t1[:], inp_bc[h][:], AF.Copy,
                        bias=0.0, scale=a1[:, b : b + 1],
                    )
                    t1s.append(t1)

            # ---- phase B matvecs: retro first (unblocks dW_fc), then pred ----
            fbn = vec_p.tile([128, NB], F32)
            retro = vec_p.tile([128, NB], F32)
            for b in range(NB):
                matvec_block(w_retro, nc.sync, "retro", spk_bc, b,
                             retro[:, b : b + 1], gate=prefetch_gate)

            a2 = vec_p.tile([128, NB], F32)   # bf*retro*surr(cur_prev)
            nc.vector.tensor_mul(a2[:], retro[:], surr_p[:])
            nc.vector.tensor_scalar(a2[:], a2[:], bf_col[:], None, ALU.mult)

            # ---- dW_fc = t1 + a2 (x) prev_inp (emitted before the pred
            # matvecs so its stores start mid-phase-B) ----
            for b in range(NB):
                for h in range(NF_IN):
                    t1 = t1s[b * NF_IN + h]
                    t2 = st2_p.tile([128, FT], F32, name="t2")
                    nc.vector.tensor_scalar(
                        t2[:], pinp_bc[h][:], a2[:, b : b + 1], None, ALU.mult
                    )
                    nc.vector.tensor_add(t2[:], t2[:], t1[:])
                    nc.sync.dma_start(
                        dwf_o.ap()[128 * b : 128 * (b + 1), FT * h : FT * (h + 1)],
                        t2[:],
                    )

            for b in range(NB):
                matvec_block(w_pred, nc.sync, "pred", spk_bc, b,
                             fbn[:, b : b + 1], gate=prefetch_gate)
            nc.gpsimd.dma_start(fbn_o.ap(), fbn[:])

            # ---- dW_pred = a0 (x) spk_full; outers split ACT/DVE ----
            for b in range(NB):
                for h in range(NF_HID):
                    dwp = dwp_p.tile([128, FT], F32, name="dwp")
                    if b % 2 == 0:
                        nc.scalar.activation(
                            dwp[:], spk_bc[h][:], AF.Copy,
                            bias=0.0, scale=a0[:, b : b + 1],
                        )
                    else:
                        nc.vector.tensor_scalar(
                            dwp[:], spk_bc[h][:], a0[:, b : b + 1], None,
                            ALU.mult,
                        )
                    nc.sync.dma_start(
                        dwp_o.ap()[128 * b : 128 * (b + 1), FT * h : FT * (h + 1)],
                        dwp[:],
                    )

    nc.compile()
    return nc


_NC_CACHE = None


def _get_program():
    global _NC_CACHE
    if _NC_CACHE is None:
        _NC_CACHE = _build_program()
    return _NC_CACHE


def _swz(v):
    # [HS] -> [128, NB] with (p, b) = v[128b + p]
    return np.ascontiguousarray(v.reshape(NB, 128).T)


def _unswz(m):
    # [128, NB] -> [HS]
    return np.ascontiguousarray(m.T).reshape(-1)


def kernel(inp, bf, W_fc, W_pred, W_retro, feedback, cur_prev, prev_inp, mem):
    inp = np.asarray(inp, np.float32)
    bf = np.asarray(bf, np.float32)
    W_fc = np.asarray(W_fc, np.float32)
    W_pred = np.asarray(W_pred, np.float32)
    W_retro = np.asarray(W_retro, np.float32)
    feedback = np.asarray(feedback, np.float32)
    cur_prev = np.asarray(cur_prev, np.float32)
    prev_inp = np.asarray(prev_inp, np.float32)
    mem = np.asarray(mem, np.float32)

    nc = _get_program()

    in_maps = []
    for c in range(NCORES):
        r = slice(HS * c, HS * (c + 1))
        in_maps.append({
            "w_fc": np.ascontiguousarray(W_fc[r]),
            "w_pred": np.ascontiguousarray(W_pred[r]),
            "w_retro": np.ascontiguousarray(W_retro[r]),
            "inp": inp.reshape(1, N_IN),
            "prev_inp": prev_inp.reshape(1, N_IN),
            "bf": bf.reshape(1, 1),
            "feedback": _swz(feedback[r]),
            "cur_prev": _swz(cur_prev[r]),
            "mem": _swz(mem[r]),
        })

    res = run_bass_kernel_spmd(nc, in_maps, list(range(NCORES))).results

    spk = np.concatenate([_unswz(res[c]["spk_out"]) for c in range(NCORES)])
    mem_out = np.concatenate([_unswz(res[c]["mem_out"]) for c in range(NCORES)])
    fb_new = np.concatenate([_unswz(res[c]["fb_new_out"]) for c in range(NCORES)])
    dw_pred = np.concatenate([res[c]["dw_pred_out"] for c in range(NCORES)], axis=0)
    dw_fc = np.concatenate([res[c]["dw_fc_out"] for c in range(NCORES)], axis=0)
    return spk, mem_out, dw_pred, dw_fc, fb_new
